# Initial kernel scaffold
#
"""Multi-head self-attention block on 8 trn2 NeuronCores.

Strategy: tensor-parallel over heads (16 heads -> 2 per core) for QKV+attention,
AllToAll of attention outputs, then each core runs the full output projection for
its 1/8 token shard. See bottom for the host-side kernel() entry point.
"""
import sys
sys.path.insert(0, "/opt/trn_rl_repo")

import numpy as np
import ml_dtypes

import concourse.bass as bass
import concourse.mybir as mybir
import concourse.tile as tile
from concourse import bacc
from concourse.bass_utils import run_bass_kernel_spmd
from concourse.masks import make_identity

# Problem shape (hardcoded per contract)
N, T, D, H = 4, 2048, 1024, 16
DK = D // H          # 64
NC = 8               # cores
HPC = H // NC        # 2 heads per core
NT = N * T           # 8192 tokens
SHARD = NT // NC     # 1024 tokens per core after A2A
TCH = 256            # token chunk for QKV projection matmuls
KT_PER_N = T // 128  # 16 key tiles per batch
QC_PER_N = T // 512  # 4 query chunks of 512 per batch

F32 = mybir.dt.float32
F32R = mybir.dt.float32r
BF16 = mybir.dt.bfloat16

FT = mybir.ActivationFunctionType


def _mmdt(ap, dt_mm):
    """Reinterpret an f32 AP as float32r for full-rate PE matmuls."""
    return ap.bitcast(dt_mm) if ap.dtype != dt_mm else ap


def build_bass():
    nc = bacc.Bacc("TRN2", target_bir_lowering=False, debug=False, num_devices=NC)

    zT = nc.dram_tensor("zT", [D, NT], F32, kind="ExternalInput")
    wq = nc.dram_tensor("wq", [D, HPC * DK], F32, kind="ExternalInput")
    wk = nc.dram_tensor("wk", [D, HPC * DK], F32, kind="ExternalInput")
    wv = nc.dram_tensor("wv", [D, HPC * DK], F32, kind="ExternalInput")
    wout = nc.dram_tensor("wout", [D, D], F32, kind="ExternalInput")
    masks = nc.dram_tensor("masks", [4, 128, 512], BF16, kind="ExternalInput")
    outT = nc.dram_tensor("outT", [D, SHARD], F32, kind="ExternalOutput")

    zT_v = zT.rearrange("(c p) t -> p c t", p=128)     # [128, 8, NT]
    wq_v = wq.rearrange("(c p) m -> p c m", p=128)     # [128, 8, 128]
    wk_v = wk.rearrange("(c p) m -> p c m", p=128)
    wv_v = wv.rearrange("(c p) m -> p c m", p=128)
    wout_v = wout.rearrange("(c p) m -> p c m", p=128)  # [128, 8, 1024]

    with tile.TileContext(nc) as tc:
        _build_body(nc, tc, zT_v, wq_v, wk_v, wv_v, wout_v, masks, outT)
    nc.compile()
    return nc


def _build_body(nc, tc, zT_v, wq_v, wk_v, wv_v, wout_v, masks, outT):
    import contextlib
    ctx = contextlib.ExitStack()
    with ctx:
        consts = ctx.enter_context(tc.tile_pool(name="consts", bufs=1))
        zpool = ctx.enter_context(tc.tile_pool(name="zpool", bufs=2))
        qkpool = ctx.enter_context(tc.tile_pool(name="qkpool", bufs=2))
        vpool = ctx.enter_context(tc.tile_pool(name="vpool", bufs=2))
        vtpool = ctx.enter_context(tc.tile_pool(name="vtpool", bufs=2))
        expool = ctx.enter_context(tc.tile_pool(name="expool", bufs=2))
        smalls = ctx.enter_context(tc.tile_pool(name="smalls", bufs=4))
        outpool = ctx.enter_context(tc.tile_pool(name="outpool", bufs=2))
        # PSUM pools: 4 (scores) + 2 (av) + 2 (mm) = 8 banks
        ps_sc = ctx.enter_context(tc.tile_pool(name="ps_sc", bufs=1, space="PSUM"))
        ps_av = ctx.enter_context(tc.tile_pool(name="ps_av", bufs=1, space="PSUM"))
        ps_mm = ctx.enter_context(tc.tile_pool(name="ps_mm", bufs=2, space="PSUM"))
        dram = ctx.enter_context(tc.tile_pool(name="dram", bufs=1, space="DRAM"))

        # ---- constants ----
        wq_sb = consts.tile([128, 8, 128], F32, tag="wq")
        wk_sb = consts.tile([128, 8, 128], F32, tag="wk")
        wv_sb = consts.tile([128, 8, 128], F32, tag="wv")
        nc.sync.dma_start(out=wq_sb, in_=wq_v)
        nc.sync.dma_start(out=wk_sb, in_=wk_v)
        nc.sync.dma_start(out=wv_sb, in_=wv_v)
        wout_sb = consts.tile([128, 8, 1024], F32, tag="wout")
        nc.sync.dma_start(out=wout_sb, in_=wout_v)
        masks_sb = consts.tile([128, 4, 512], BF16, tag="masks")
        nc.sync.dma_start(out=masks_sb, in_=masks.rearrange("d p m -> p d m"))
        ident_sb = consts.tile([128, 128], BF16, tag="ident")
        make_identity(nc, ident_sb)
        ones_sb = consts.tile([1, 64], F32, tag="ones")
        nc.vector.memset(ones_sb, 1.0)
        # attention output (transposed): rows = 2 local heads x 64, cols = all tokens
        attnT = consts.tile([128, NT], BF16, tag="attnT")

        a2a_in = dram.tile([NC, 128, SHARD], BF16, tag="a2a_in")
        a2a_out = dram.tile([NC, 128, SHARD], BF16, tag="a2a_out",
                            addr_space="Shared")

        for n in range(N):
            tok0 = n * T
            # ---- QKV projection for batch n ----
            qt = qkpool.tile([128, T], F32, tag="qt")
            kt_sb = qkpool.tile([128, T], F32, tag="kt")
            # V stored token-major per 128-token tile: [v_h | 1] slots of 65 cols
            vsb = vpool.tile([128, KT_PER_N, HPC, 65], BF16, tag="v")
            nc.gpsimd.memset(vsb, 1.0)
            for tci in range(T // TCH):
                zch = zpool.tile([128, 8, TCH], F32, tag="z")
                nc.sync.dma_start(
                    out=zch, in_=zT_v[:, :, tok0 + tci * TCH: tok0 + (tci + 1) * TCH])
                for w_sb, dst in ((wq_sb, qt), (wk_sb, kt_sb)):
                    ps = ps_mm.tile([128, 512], F32, tag="mm")
                    for dc in range(8):
                        nc.tensor.matmul(
                            ps[:, :TCH], lhsT=_mmdt(w_sb[:, dc, :], F32R),
                            rhs=_mmdt(zch[:, dc, :], F32R),
                            start=(dc == 0), stop=(dc == 7))
                    nc.vector.tensor_copy(dst[:, tci * TCH:(tci + 1) * TCH],
                                          ps[:, :TCH])
                # V.T chunk then transpose into token-major slots
                ps = ps_mm.tile([128, 512], F32, tag="mm")
                for dc in range(8):
                    nc.tensor.matmul(
                        ps[:, :TCH], lhsT=_mmdt(wv_sb[:, dc, :], F32R),
                        rhs=_mmdt(zch[:, dc, :], F32R),
                        start=(dc == 0), stop=(dc == 7))
                vt_bf = vtpool.tile([128, TCH], BF16, tag="vt")
                nc.vector.tensor_copy(vt_bf, ps[:, :TCH])
                for sub in range(TCH // 128):
                    kt_idx = (tci * TCH) // 128 + sub
                    for h in range(HPC):
                        pst = ps_mm.tile([128, 64], BF16, tag="mm")
                        nc.tensor.transpose(
                            pst, vt_bf[h * 64:(h + 1) * 64, sub * 128:(sub + 1) * 128],
                            ident_sb[0:64, 0:64])
                        nc.vector.tensor_copy(vsb[:, kt_idx, h, 0:64], pst)

            # ---- attention for batch n ----
            for qc in range(QC_PER_N):
                q0 = qc * 512
                n_kt = 4 * qc + 4
                av = [ps_av.tile([65, 512], F32, tag=f"av{h}") for h in range(HPC)]
                for g in range(n_kt // 2):
                    kts = (2 * g, 2 * g + 1)
                    sc = ps_sc.tile([128, 4, 512], F32, tag="sc")
                    for h in range(HPC):
                        for j, kt in enumerate(kts):
                            nc.tensor.matmul(
                                sc[:, 2 * h + j, :],
                                lhsT=_mmdt(kt_sb[h * 64:(h + 1) * 64,
                                                 kt * 128:(kt + 1) * 128], F32R),
                                rhs=_mmdt(qt[h * 64:(h + 1) * 64, q0:q0 + 512], F32R),
                                start=True, stop=True)
                    ex = expool.tile([128, 4, 512], BF16, tag="ex")
                    nc.scalar.activation(ex, sc, FT.Exp)
                    for h in range(HPC):
                        for j, kt in enumerate(kts):
                            if kt >= 4 * qc:
                                d = kt - 4 * qc
                                nc.vector.tensor_mul(
                                    ex[:, 2 * h + j, :], ex[:, 2 * h + j, :],
                                    masks_sb[:, d, :])
                    for h in range(HPC):
                        for j, kt in enumerate(kts):
                            nc.tensor.matmul(
                                av[h], lhsT=vsb[:, kt, h, :], rhs=ex[:, 2 * h + j, :],
                                start=(kt == 0), stop=(kt == n_kt - 1))
                for h in range(HPC):
                    rd = smalls.tile([1, 512], F32, tag="rd")
                    nc.vector.reciprocal(rd, av[h][64:65, :])
                    bc = ps_mm.tile([64, 512], F32, tag="mm")
                    nc.tensor.matmul(bc, lhsT=_mmdt(ones_sb, F32R),
                                     rhs=_mmdt(rd, F32R), start=True, stop=True)
                    nc.vector.tensor_mul(
                        attnT[h * 64:(h + 1) * 64, tok0 + q0: tok0 + q0 + 512],
                        av[h][0:64, :], bc)

        # ---- AllToAll: exchange token shards ----
        for j in range(NC):
            nc.sync.dma_start(out=a2a_in[j], in_=attnT[:, j * SHARD:(j + 1) * SHARD])
        nc.gpsimd.collective_compute(
            "AllToAll", mybir.AluOpType.bypass,
            replica_groups=[list(range(NC))],
            ins=[a2a_in.opt()], outs=[a2a_out.opt()])
        rhs_bf = consts.tile([128, NC, SHARD], BF16, tag="rhs_bf")
        for j in range(NC):
            nc.sync.dma_start(out=rhs_bf[:, j, :], in_=a2a_out[j])
        rhs_f = consts.tile([128, NC, SHARD], F32, tag="rhs_f")
        nc.vector.tensor_copy(rhs_f, rhs_bf)

        # ---- output projection for my token shard ----
        for ot in range(8):
            for tc2 in range(SHARD // 512):
                ps = ps_mm.tile([128, 512], F32, tag="mm")
                for j in range(8):
                    nc.tensor.matmul(
                        ps, lhsT=_mmdt(wout_sb[:, j, ot * 128:(ot + 1) * 128], F32R),
                        rhs=_mmdt(rhs_f[:, j, tc2 * 512:(tc2 + 1) * 512], F32R),
                        start=(j == 0), stop=(j == 7))
                ob = outpool.tile([128, 512], F32, tag="ob")
                nc.vector.tensor_copy(ob, ps)
                nc.sync.dma_start(
                    out=outT[ot * 128:(ot + 1) * 128, tc2 * 512:(tc2 + 1) * 512],
                    in_=ob)


_NC_CACHE = None


def _get_nc():
    global _NC_CACHE
    if _NC_CACHE is None:
        _NC_CACHE = build_bass()
    return _NC_CACHE


def _make_masks():
    r = np.arange(128)[:, None]
    c = np.arange(512)[None, :]
    m = np.stack([(c >= 128 * d + r) for d in range(4)]).astype(np.float32)
    return m.astype(ml_dtypes.bfloat16)


def _prepare_in_maps(z, Wqkv, Wout):
    zT = np.ascontiguousarray(z.reshape(NT, D).T).astype(np.float32)
    scale = DK ** -0.5
    Wq = (Wqkv[:, :D] * scale).reshape(D, H, DK)
    Wk = Wqkv[:, D:2 * D].reshape(D, H, DK)
    Wv = Wqkv[:, 2 * D:].reshape(D, H, DK)
    masks = _make_masks()
    in_maps = []
    for core in range(NC):
        h0 = HPC * core
        wq_c = np.ascontiguousarray(
            Wq[:, h0:h0 + HPC, :].reshape(D, HPC * DK)).astype(np.float32)
        wk_c = np.ascontiguousarray(
            Wk[:, h0:h0 + HPC, :].reshape(D, HPC * DK)).astype(np.float32)
        wv_c = np.ascontiguousarray(
            Wv[:, h0:h0 + HPC, :].reshape(D, HPC * DK)).astype(np.float32)
        in_maps.append({
            "zT": zT, "wq": wq_c, "wk": wk_c, "wv": wv_c,
            "wout": np.ascontiguousarray(Wout).astype(np.float32),
            "masks": masks,
        })
    return in_maps


def _run(z, Wqkv, Wout, trace=False):
    nc = _get_nc()
    in_maps = _prepare_in_maps(z, Wqkv, Wout)
    res = run_bass_kernel_spmd(nc, in_maps, core_ids=list(range(NC)), trace=trace)
    out = np.empty((NT, D), dtype=np.float32)
    for core in range(NC):
        shard = res.results[core]["outT"]  # [D, SHARD]
        out[core * SHARD:(core + 1) * SHARD, :] = shard.T
    return out.reshape(N, T, D), res


def kernel(z, Wqkv, Wout):
    out, _ = _run(np.asarray(z), np.asarray(Wqkv), np.asarray(Wout))
    return out


# revision 27
# speedup vs baseline: 1.5388x; 1.5388x over previous
"""Multi-head self-attention block on 8 trn2 NeuronCores.

Strategy: tensor-parallel over heads (16 heads -> 2 per core) for QKV+attention,
AllToAll of attention outputs, then each core runs the full output projection for
its 1/8 token shard. See bottom for the host-side kernel() entry point.
"""
import sys
sys.path.insert(0, "/opt/trn_rl_repo")

import numpy as np
import ml_dtypes

import concourse.bass as bass
import concourse.mybir as mybir
import concourse.tile as tile
from concourse import bacc
from concourse.bass_utils import run_bass_kernel_spmd
from concourse.masks import make_identity

# Problem shape (hardcoded per contract)
N, T, D, H = 4, 2048, 1024, 16
DK = D // H          # 64
NC = 8               # cores
HPC = H // NC        # 2 heads per core
NT = N * T           # 8192 tokens
SHARD = NT // NC     # 1024 tokens per core after A2A
TCH = 512            # token chunk for QKV projection matmuls
KT_PER_N = T // 128  # 16 key tiles per batch
QC_PER_N = T // 512  # 4 query chunks of 512 per batch

F32 = mybir.dt.float32
F32R = mybir.dt.float32r
BF16 = mybir.dt.bfloat16

FT = mybir.ActivationFunctionType


def _mmdt(ap, dt_mm):
    """Reinterpret an f32 AP as float32r for full-rate PE matmuls."""
    return ap.bitcast(dt_mm) if ap.dtype != dt_mm else ap


def build_bass():
    nc = bacc.Bacc("TRN2", target_bir_lowering=False, debug=False, num_devices=NC)

    zT = nc.dram_tensor("zT", [D, NT], F32R, kind="ExternalInput")
    wq = nc.dram_tensor("wq", [D, HPC * DK], F32R, kind="ExternalInput")
    wk = nc.dram_tensor("wk", [D, HPC * DK], F32R, kind="ExternalInput")
    wv = nc.dram_tensor("wv", [D, HPC * DK], F32R, kind="ExternalInput")
    wout = nc.dram_tensor("wout", [D, D], F32R, kind="ExternalInput")
    masks = nc.dram_tensor("masks", [4, 128, 512], BF16, kind="ExternalInput")
    ones = nc.dram_tensor("ones", [1, 64], F32R, kind="ExternalInput")
    outT = nc.dram_tensor("outT", [D, SHARD], F32, kind="ExternalOutput")

    zT_v = zT.rearrange("(c p) t -> p c t", p=128)     # [128, 8, NT]
    wq_v = wq.rearrange("(c p) m -> p c m", p=128)     # [128, 8, 128]
    wk_v = wk.rearrange("(c p) m -> p c m", p=128)
    wv_v = wv.rearrange("(c p) m -> p c m", p=128)
    wout_v = wout.rearrange("(c p) m -> p c m", p=128)  # [128, 8, 1024]

    with tile.TileContext(nc) as tc:
        _build_body(nc, tc, zT_v, wq_v, wk_v, wv_v, wout_v, masks, ones, outT)
    nc.compile()
    return nc


def _build_body(nc, tc, zT_v, wq_v, wk_v, wv_v, wout_v, masks, ones, outT):
    import contextlib
    ctx = contextlib.ExitStack()
    with ctx:
        consts = ctx.enter_context(tc.tile_pool(name="consts", bufs=1))
        zpool = ctx.enter_context(tc.tile_pool(name="zpool", bufs=2))
        qkpool = ctx.enter_context(tc.tile_pool(name="qkpool", bufs=2))
        vpool = ctx.enter_context(tc.tile_pool(name="vpool", bufs=2))
        vtpool = ctx.enter_context(tc.tile_pool(name="vtpool", bufs=2))
        expool = ctx.enter_context(tc.tile_pool(name="expool", bufs=2))
        smalls = ctx.enter_context(tc.tile_pool(name="smalls", bufs=4))
        outpool = ctx.enter_context(tc.tile_pool(name="outpool", bufs=2))
        # PSUM pools: 4 (scores) + 2 (av) + 2 (mm) = 8 banks
        ps_sc = ctx.enter_context(tc.tile_pool(name="ps_sc", bufs=1, space="PSUM"))
        ps_av = ctx.enter_context(tc.tile_pool(name="ps_av", bufs=1, space="PSUM"))
        ps_mm = ctx.enter_context(tc.tile_pool(name="ps_mm", bufs=2, space="PSUM"))
        dram = ctx.enter_context(tc.tile_pool(name="dram", bufs=1, space="DRAM"))

        # ---- constants ----
        wq_sb = consts.tile([128, 8, 128], F32R, tag="wq")
        wk_sb = consts.tile([128, 8, 128], F32R, tag="wk")
        wv_sb = consts.tile([128, 8, 128], F32R, tag="wv")
        nc.sync.dma_start(out=wq_sb, in_=wq_v)
        nc.sync.dma_start(out=wk_sb, in_=wk_v)
        nc.sync.dma_start(out=wv_sb, in_=wv_v)
        wout_sb = consts.tile([128, 8, 1024], F32R, tag="wout")
        nc.gpsimd.dma_start(out=wout_sb, in_=wout_v)
        masks_sb = consts.tile([128, 4, 512], BF16, tag="masks")
        nc.gpsimd.dma_start(out=masks_sb, in_=masks.rearrange("d p m -> p d m"))
        ident_sb = consts.tile([128, 128], BF16, tag="ident")
        make_identity(nc, ident_sb)
        ones_sb = consts.tile([1, 64], F32R, tag="ones")
        nc.sync.dma_start(out=ones_sb, in_=ones[:, :])
        # attention output (transposed): rows = 2 local heads x 64, cols = all tokens
        attnT = consts.tile([128, NT], BF16, tag="attnT")


        pending = []

        def _proj_consume(items):
            # items: list of (half_index g, a2aout [NC, 128, 128]) — 1 or 2
            nh = len(items)
            w = 128 * nh
            rhs_bf = smalls.tile([128, NC, nh, 128], BF16, tag="rhs_bf", bufs=2,
                                 name="rhsbf")
            for i in range(NC):
                for s, (_, a2aout) in enumerate(items):
                    nc.gpsimd.dma_start(out=rhs_bf[:, i, s, :], in_=a2aout[i])
            rhs_f = smalls.tile([128, NC, nh, 128], F32R, tag="rhs_f", bufs=2,
                                name="rhsf")
            nc.vector.tensor_copy(rhs_f, rhs_bf)
            for ot in range(8):
                ps = ps_mm.tile([128, 512], F32, tag="mm", name="psproj")
                for i in range(NC):
                    nc.tensor.matmul(
                        ps[:, :w],
                        lhsT=wout_sb[:, i, ot * 128:(ot + 1) * 128],
                        rhs=rhs_f[:, i, :, :], start=(i == 0), stop=(i == NC - 1))
                ob = outpool.tile([128, 256], F32, tag="ob")
                nc.vector.tensor_copy(ob[:, :w], ps[:, :w])
                for s, (g, _) in enumerate(items):
                    nc.sync.dma_start(
                        out=outT[ot * 128:(ot + 1) * 128, g * 128:(g + 1) * 128],
                        in_=ob[:, s * 128:(s + 1) * 128])

        qkv_state = {}

        def _qkv_start(n):
            tok0 = n * T
            qt = qkpool.tile([128, T], F32R, tag="qt", name=f"qt{n}")
            kt_sb = qkpool.tile([128, T], F32R, tag="kt", name=f"kt{n}")
            vsb = vpool.tile([128, KT_PER_N, HPC, 65], BF16, tag="v",
                             name=f"v{n}")
            nc.vector.memset(vsb[:, :, :, 64:65], 1.0)
            qkv_state[n] = (qt, kt_sb, vsb)

        def _qkv_chunk(n, tci):
            tok0 = n * T
            qt, kt_sb, vsb = qkv_state[n]
            zch = zpool.tile([128, 8, TCH], F32R, tag="z", name="zch")
            nc.sync.dma_start(
                out=zch, in_=zT_v[:, :, tok0 + tci * TCH: tok0 + (tci + 1) * TCH])
            for w_sb, dst in ((wq_sb, qt), (wk_sb, kt_sb)):
                ps = ps_mm.tile([128, 512], F32, tag="mm", name="psqk")
                for dc in range(8):
                    nc.tensor.matmul(
                        ps[:, :TCH], lhsT=w_sb[:, dc, :], rhs=zch[:, dc, :],
                        start=(dc == 0), stop=(dc == 7))
                nc.vector.tensor_copy(dst[:, tci * TCH:(tci + 1) * TCH],
                                      ps[:, :TCH])
            ps = ps_mm.tile([128, 512], F32, tag="mm", name="psv")
            for dc in range(8):
                nc.tensor.matmul(
                    ps[:, :TCH], lhsT=wv_sb[:, dc, :], rhs=zch[:, dc, :],
                    start=(dc == 0), stop=(dc == 7))
            vt_bf = vtpool.tile([128, TCH], BF16, tag="vt", name="vtbf")
            nc.vector.tensor_copy(vt_bf, ps[:, :TCH])
            for sub in range(TCH // 128):
                kt_idx = (tci * TCH) // 128 + sub
                for h in range(HPC):
                    pst = ps_mm.tile([128, 64], BF16, tag="mm", name="pst")
                    nc.tensor.transpose(
                        pst, vt_bf[h * 64:(h + 1) * 64, sub * 128:(sub + 1) * 128],
                        ident_sb[h * 64:(h + 1) * 64, h * 64:(h + 1) * 64])
                    nc.vector.tensor_copy(vsb[:, kt_idx, h, 0:64], pst)

        def _attn_qc(n, qc):
            tok0 = n * T
            qt, kt_sb, vsb = qkv_state[n]
            q0 = qc * 512
            n_kt = 4 * qc + 4
            av = [ps_av.tile([65, 512], F32, tag=f"av{h}", name=f"av{h}")
                  for h in range(HPC)]
            for kt in range(n_kt):
                # columns [0, s) of this kt row-block are fully causal-masked
                d = kt - 4 * qc
                s = 128 * d if d > 0 else 0
                if 512 - s == 128:
                    s = 256  # f32r below N=256 runs 4 cyc/row; keep N>=256
                sc = ps_sc.tile([128, 2, 512], F32, tag="sc", name="sc", bufs=2)
                for h in range(HPC):
                    nc.tensor.matmul(
                        sc[:, h, s:],
                        lhsT=kt_sb[h * 64:(h + 1) * 64,
                                   kt * 128:(kt + 1) * 128],
                        rhs=qt[h * 64:(h + 1) * 64, q0 + s:q0 + 512],
                        start=True, stop=True)
                ex = expool.tile([128, 2, 512], BF16, tag="ex", name="ex",
                                 bufs=3)
                nc.scalar.activation(ex[:, :, s:], sc[:, :, s:], FT.Exp)
                if s > 0:
                    nc.vector.memset(ex[:, :, :s], 0.0)
                if d >= 0:
                    for h in range(HPC):
                        nc.vector.tensor_mul(
                            ex[:, h, s:], ex[:, h, s:], masks_sb[:, d, s:])
                for h in range(HPC):
                    nc.tensor.matmul(
                        av[h][:, s:], lhsT=vsb[:, kt, h, :], rhs=ex[:, h, s:],
                        start=(kt == 0), stop=(kt == n_kt - 1))
            for h in range(HPC):
                av_sb = smalls.tile([64, 512], F32, tag=f"avs{h}",
                                    name=f"avs{h}", bufs=2)
                nc.vector.tensor_copy(av_sb, av[h][0:64, :])
                d_sb = smalls.tile([1, 512], F32, tag="d_sb", bufs=2, name="dsb")
                nc.vector.tensor_copy(d_sb, av[h][64:65, :])
                rd = smalls.tile([1, 512], F32, tag="rd", bufs=2, name="rd")
                nc.vector.reciprocal_approx_fast(rd, d_sb)
                rdb = smalls.tile([64, 512], F32, tag="rdb", bufs=2, name="rdb")
                dscr = dram.tile([1, 512], F32, tag="dscr", bufs=2, name="dscr")
                nc.sync.dma_start(out=dscr, in_=rd)
                nc.sync.dma_start(out=rdb, in_=dscr.to_broadcast([64, 512]))
                nc.vector.tensor_mul(
                    attnT[h * 64:(h + 1) * 64, tok0 + q0: tok0 + q0 + 512],
                    av_sb, rdb)

        def _a2a_issue(g):
            # half-batch g covers tokens [g*1024, (g+1)*1024); core i owns
            # 128 tokens at offset i*128 within it
            base = g * 1024
            a2ain = dram.tile([NC, 128, 128], BF16, tag="a2ain", bufs=4,
                              name=f"a2ain{g}")
            a2aout = dram.tile([NC, 128, 128], BF16, tag="a2aout", bufs=4,
                               name=f"a2aout{g}")
            for i in range(NC):
                nc.sync.dma_start(
                    out=a2ain[i],
                    in_=attnT[:, base + i * 128: base + (i + 1) * 128])
            nc.gpsimd.collective_compute(
                "AllToAll", mybir.AluOpType.bypass,
                replica_groups=[list(range(NC))],
                ins=[a2ain.opt()], outs=[a2aout.opt()])
            pending.append((g, a2aout))

        _qkv_start(0)
        for tci in range(T // TCH):
            _qkv_chunk(0, tci)
        for n in range(N):
            if n + 1 < N:
                _qkv_start(n + 1)
            for qc in range(QC_PER_N):
                _attn_qc(n, qc)
                if n + 1 < N:
                    _qkv_chunk(n + 1, qc)
                if qc == 1 or qc == 3:
                    _a2a_issue(2 * n + qc // 2)
                elif len(pending) >= 3:
                    _proj_consume([pending.pop(0), pending.pop(0)])
        while pending:
            items = [pending.pop(0)]
            if pending:
                items.append(pending.pop(0))
            _proj_consume(items)


_NC_CACHE = None


def _get_nc():
    global _NC_CACHE
    if _NC_CACHE is None:
        _NC_CACHE = build_bass()
    return _NC_CACHE


def _make_masks():
    r = np.arange(128)[:, None]
    c = np.arange(512)[None, :]
    m = np.stack([(c >= 128 * d + r) for d in range(4)]).astype(np.float32)
    return m.astype(ml_dtypes.bfloat16)


def _prepare_in_maps(z, Wqkv, Wout):
    zT = np.ascontiguousarray(z.reshape(NT, D).T).astype(np.float32)
    scale = DK ** -0.5
    Wq = (Wqkv[:, :D] * scale).reshape(D, H, DK)
    Wk = Wqkv[:, D:2 * D].reshape(D, H, DK)
    Wv = Wqkv[:, 2 * D:].reshape(D, H, DK)
    masks = _make_masks()
    in_maps = []
    for core in range(NC):
        h0 = HPC * core
        wq_c = np.ascontiguousarray(
            Wq[:, h0:h0 + HPC, :].reshape(D, HPC * DK)).astype(np.float32)
        wk_c = np.ascontiguousarray(
            Wk[:, h0:h0 + HPC, :].reshape(D, HPC * DK)).astype(np.float32)
        wv_c = np.ascontiguousarray(
            Wv[:, h0:h0 + HPC, :].reshape(D, HPC * DK)).astype(np.float32)
        in_maps.append({
            "zT": zT, "wq": wq_c, "wk": wk_c, "wv": wv_c,
            "ones": np.ones((1, 64), dtype=np.float32),
            "wout": np.ascontiguousarray(Wout).astype(np.float32),
            "masks": masks,
        })
    return in_maps


def _run(z, Wqkv, Wout, trace=False):
    nc = _get_nc()
    in_maps = _prepare_in_maps(z, Wqkv, Wout)
    res = run_bass_kernel_spmd(nc, in_maps, core_ids=list(range(NC)), trace=trace)
    out = np.empty((NT, D), dtype=np.float32)
    for core in range(NC):
        shard = res.results[core]["outT"].reshape(D, NT // 1024, 128)
        for g in range(NT // 1024):
            s0 = g * 1024 + core * 128
            out[s0:s0 + 128, :] = shard[:, g, :].T
    return out.reshape(N, T, D), res


def kernel(z, Wqkv, Wout):
    out, _ = _run(np.asarray(z), np.asarray(Wqkv), np.asarray(Wout))
    return out
